# revision 8
# baseline (speedup 1.0000x reference)
"""Multi-head attention TRN2 kernel: 8 cores = 4 batch x 2 head-groups.

Per core (b = core//2, g = core%2): attention for batch b, heads [8g, 8g+8),
producing the transposed partial output projection. Host sums the two
head-group partials per batch and adds (bo + Wo @ bv) once.

Precision: fp16 for the Q/K chain (x, W, Q^T, K^T, QK matmul — 10-bit
mantissa keeps score error small on std-8 scores), bf16 for the V/P/C/Wo
chain (range needed for exp(s-45) ~ 1e-20..1e16). PSUM f32.

Layouts (per core, host-prepped):
  xq/xk : x^T   [1024 d, 2048 t] f16
  xv    : x^T   [1024 d, 2048 t] bf16
  wq/wk : W_g^T [1024 d, 512 j]  f16
  wv    : W_g^T [1024 d, 512 j]  bf16
  wo    : Wo_g^T [512 c, 1024 j] bf16
  out   : OUT^T partial [1024 j, 2048 t] f32

The scalar engine's exp (256 ACTIVATEs of [128,1024], ~18.5us per
(pair,q-chunk)) is the co-critical resource next to the PE, so the schedule
is built around starting it early and never starving it: K proj, Q proj
(t-chunk 0 only needed), then QK blocks for q-chunk 0 begin immediately
while V proj and the remaining Q proj chunks fill the PE between them.
pt (exp output) is triple-buffered so QK(n) never waits on AV(n-2)'s
reads. Steady state runs a depth-3 software pipeline av(n-3); qk(n);
with out-proj for q-chunk q slotted after qk(4q+6) to cover the last
pair's normalization latency. Weight tiles for wk/wq/wv rotate through one
double-buffered slot (their lifetimes are disjoint); ct rotates per
q-chunk; x chunks load in half-d-blocks, triple buffered.
"""

import numpy as np
import ml_dtypes

D = 1024          # d_model
L = 2048          # sequence length
B = 4             # batch
HG = 512          # head-group width (8 heads x 64)
NCORES = 8
EXP_BIAS = -45.0  # softmax shift: exp(s-45); cancels in normalization

NT = 4            # token chunks of 512
TC = L // NT      # 512
NDB = D // 128    # 8 d-model blocks
NP = 4            # head pairs per group
NKB = L // 128    # 16 key blocks

_COMPILED = None
LAST_RESULT = None


def _build():
    import concourse.bacc as bacc
    import concourse.mybir as mybir
    import concourse.tile as tile

    f32 = mybir.dt.float32
    f16 = mybir.dt.float16
    bf16 = mybir.dt.bfloat16
    EXP = mybir.ActivationFunctionType.Exp
    MUL = mybir.AluOpType.mult

    nc = bacc.Bacc()

    xq = nc.declare_dram_parameter("xq", [D, L], f16, isOutput=False)
    xk = nc.declare_dram_parameter("xk", [D, L], f16, isOutput=False)
    xv = nc.declare_dram_parameter("xv", [D, L], bf16, isOutput=False)
    wq = nc.declare_dram_parameter("wq", [D, HG], f16, isOutput=False)
    wk = nc.declare_dram_parameter("wk", [D, HG], f16, isOutput=False)
    wv = nc.declare_dram_parameter("wv", [D, HG], bf16, isOutput=False)
    wo = nc.declare_dram_parameter("wo", [HG, D], bf16, isOutput=False)
    bq = nc.declare_dram_parameter("bq", [HG], f32, isOutput=False)
    out = nc.declare_dram_parameter("out", [D, L], f32, isOutput=True)

    out_v = out.rearrange("(ob p) (n t) -> ob p n t", p=128, t=TC)

    with tile.TileContext(nc) as tc:
        with tc.tile_pool(name="res", bufs=1) as res, tc.tile_pool(
            name="pa", bufs=1
        ) as pa, tc.tile_pool(name="psum", bufs=1, space="PSUM") as psum:
            # ---- resident tiles ----
            kt_sb = res.tile([128, NP, L], f16)
            qt_sb = res.tile([128, NP, L], f16)
            wo_sb = res.tile([128, NP, D], bf16)
            bq_sb = res.tile([128, NP], f32)
            bias_exp = res.tile([128, 1], f32)
            nc.vector.memset(bias_exp[:], EXP_BIAS)

            # V stationary: per (kb, pair): [V_even(64) | 1 | V_odd(64) | 1]
            # even AV uses cols 0:65, odd cols 65:130; both land at PSUM
            # partitions 0:65 with the rowsum at partition 64.
            v_sb = res.tile([128, NKB, NP, 130], bf16)
            nc.vector.memset(v_sb[:, :, :, 64:65], 1.0)
            nc.vector.memset(v_sb[:, :, :, 129:130], 1.0)

            def wtile(dt):
                return pa.tile([128, NDB, HG], dt, name="w", tag="w", bufs=2)

            def load_x_half(src, t, h, dt):
                xt = pa.tile([128, NDB // 2, TC], dt, name="xt", tag="xt", bufs=3)
                nc.sync.dma_start(
                    out=xt[:],
                    in_=src.rearrange("(db p) (n t) -> p db n t", p=128, t=TC)[
                        :, 4 * h : 4 * h + 4, t
                    ],
                )
                return xt

            def emit_kqproj(src, w_sb, t, is_q):
                xh = [load_x_half(src, t, h, f16) for h in range(2)]
                for jb in range(NP):
                    ps = psum.tile([128, TC], f32, name="acc", tag="accu", bufs=2)
                    for db in range(NDB):
                        nc.tensor.matmul(
                            ps[:],
                            w_sb[:, db, jb * 128 : (jb + 1) * 128],
                            xh[db // 4][:, db % 4, :],
                            start=(db == 0),
                            stop=(db == NDB - 1),
                        )
                    if is_q:
                        nc.vector.tensor_scalar_add(
                            qt_sb[:, jb, t * TC : (t + 1) * TC],
                            ps[:],
                            bq_sb[:, jb : jb + 1],
                        )
                    else:
                        nc.vector.tensor_copy(
                            kt_sb[:, jb, t * TC : (t + 1) * TC], ps[:]
                        )

            def emit_vproj(wv_sb, t):
                xh = [load_x_half(xv, t, h, bf16) for h in range(2)]
                for tb in range(4):
                    kb = t * 4 + tb
                    ps = psum.tile(
                        [128, NP, 128], f32, name="acc", tag="accu", bufs=2
                    )
                    for db in range(NDB):
                        nc.tensor.matmul(
                            ps[:],
                            xh[db // 4][:, db % 4, tb * 128 : (tb + 1) * 128],
                            wv_sb[:, db, :],
                            start=(db == 0),
                            stop=(db == NDB - 1),
                        )
                    nc.vector.tensor_copy(v_sb[:, kb, :, 0:64], ps[:, :, 0:64])
                    nc.vector.tensor_copy(v_sb[:, kb, :, 65:129], ps[:, :, 64:128])

            def emit_qk(p, q):
                pt = pa.tile([128, NKB, 2, TC], bf16, name="pt", tag="pt", bufs=3)
                qsl = slice(q * TC, (q + 1) * TC)
                for kb in range(NKB):
                    ps_s = psum.tile(
                        [128, 2, TC], f32, name="ps_s", tag="ps_s", bufs=2
                    )
                    nc.tensor.matmul(
                        ps_s[:, 0, :],
                        kt_sb[0:64, p, kb * 128 : (kb + 1) * 128],
                        qt_sb[0:64, p, qsl],
                        start=True,
                        stop=True,
                    )
                    nc.tensor.matmul(
                        ps_s[:, 1, :],
                        kt_sb[64:128, p, kb * 128 : (kb + 1) * 128],
                        qt_sb[64:128, p, qsl],
                        start=True,
                        stop=True,
                    )
                    nc.scalar.activation(
                        pt[:, kb, :, :], ps_s[:], EXP, bias=bias_exp[:], scale=1.0
                    )
                return pt

            def emit_av(p, ct, pt):
                ps_u = psum.tile([128, 2, TC], f32, name="ps_u", tag="accu", bufs=2)
                for kb in range(NKB):
                    nc.tensor.matmul(
                        ps_u[0:65, 0, :],
                        v_sb[:, kb, p, 0:65],
                        pt[:, kb, 0, :],
                        start=(kb == 0),
                        stop=(kb == NKB - 1),
                    )
                    nc.tensor.matmul(
                        ps_u[0:65, 1, :],
                        v_sb[:, kb, p, 65:130],
                        pt[:, kb, 1, :],
                        start=(kb == 0),
                        stop=(kb == NKB - 1),
                    )
                # normalize: ct = U^T * (1/r); odd head computed at 0:64 then
                # DMA-shifted to partitions 64:128
                rr = pa.tile([1, 2, TC], f32, name="rr", tag="rr", bufs=2)
                nc.vector.reciprocal(rr[:], ps_u[64:65, :, :])
                rb = pa.tile([64, 2, TC], f32, name="rb", tag="rb", bufs=1)
                nc.gpsimd.partition_broadcast(rb[:, 0, :], rr[:, 0, :], channels=64)
                nc.gpsimd.partition_broadcast(rb[:, 1, :], rr[:, 1, :], channels=64)
                nc.vector.tensor_tensor(
                    out=ct[0:64, p, :],
                    in0=ps_u[0:64, 0, :],
                    in1=rb[:, 0, :],
                    op=MUL,
                )
                ct_o = pa.tile([64, TC], bf16, name="ct_o", tag="ct_o", bufs=2)
                nc.vector.tensor_tensor(
                    out=ct_o[:], in0=ps_u[0:64, 1, :], in1=rb[:, 1, :], op=MUL
                )
                nc.sync.dma_start(out=ct[64:128, p, :], in_=ct_o[:])

            def emit_outproj(q, ct):
                for ob in range(NDB):
                    ps = psum.tile([128, TC], f32, name="po", tag="accu", bufs=2)
                    for p in range(NP):
                        nc.tensor.matmul(
                            ps[:],
                            wo_sb[:, p, ob * 128 : (ob + 1) * 128],
                            ct[:, p, :],
                            start=(p == 0),
                            stop=(p == NP - 1),
                        )
                    o_sb = pa.tile([128, TC], f32, name="o_sb", tag="o_sb", bufs=1)
                    nc.vector.tensor_copy(o_sb[:], ps[:])
                    nc.sync.dma_start(out=out_v[ob, :, q], in_=o_sb[:])

            # ---- schedule ----
            wk_sb = wtile(f16)
            nc.sync.dma_start(
                out=wk_sb[:], in_=wk.rearrange("(db p) j -> p db j", p=128)
            )
            emit_kqproj(xk, wk_sb, 0, False)
            wq_sb = wtile(f16)
            nc.sync.dma_start(
                out=wq_sb[:], in_=wq.rearrange("(db p) j -> p db j", p=128)
            )
            nc.sync.dma_start(out=bq_sb[:], in_=bq.rearrange("(jb p) -> p jb", p=128))
            for t in range(1, NT):
                emit_kqproj(xk, wk_sb, t, False)
            emit_kqproj(xq, wq_sb, 0, True)

            pts = {}
            cts = {}

            def new_ct():
                return pa.tile([128, NP, TC], bf16, name="ct", tag="ct", bufs=2)

            pts[0] = emit_qk(0, 0)
            wv_sb = wtile(bf16)
            nc.sync.dma_start(
                out=wv_sb[:], in_=wv.rearrange("(db p) j -> p db j", p=128)
            )
            emit_vproj(wv_sb, 0)
            pts[1] = emit_qk(1, 0)
            emit_vproj(wv_sb, 1)
            emit_kqproj(xq, wq_sb, 1, True)
            emit_vproj(wv_sb, 2)
            emit_kqproj(xq, wq_sb, 2, True)
            emit_vproj(wv_sb, 3)
            emit_kqproj(xq, wq_sb, 3, True)
            pts[2] = emit_qk(2, 0)
            nc.sync.dma_start(
                out=wo_sb[:], in_=wo.rearrange("(cb p) j -> p cb j", p=128)
            )

            for n in range(3, 16):
                m = n - 3
                if m % 4 == 0:
                    cts[m // 4] = new_ct()
                emit_av(m % 4, cts[m // 4], pts.pop(m))
                pts[n] = emit_qk(n % 4, n // 4)
                if n % 4 == 2 and n > 3:
                    emit_outproj(n // 4 - 1, cts[n // 4 - 1])
            for m in (13, 14, 15):
                if m % 4 == 0:
                    cts[m // 4] = new_ct()
                emit_av(m % 4, cts[m // 4], pts.pop(m))
            emit_outproj(3, cts[3])

    nc.compile()
    return nc


def _get_compiled():
    global _COMPILED
    if _COMPILED is None:
        _COMPILED = _build()
    return _COMPILED


def kernel(q, k, v, Wq, bq, Wk, bk, Wv, bv, Wo, bo):
    global LAST_RESULT
    from concourse.bass_utils import run_bass_kernel_spmd

    nc = _get_compiled()

    f16 = np.float16
    bf = ml_dtypes.bfloat16
    q = np.asarray(q, dtype=np.float32)
    k = np.asarray(k, dtype=np.float32)
    v = np.asarray(v, dtype=np.float32)
    Wq = np.asarray(Wq, dtype=np.float32)
    Wk = np.asarray(Wk, dtype=np.float32)
    Wv = np.asarray(Wv, dtype=np.float32)
    Wo = np.asarray(Wo, dtype=np.float32)
    bq = np.asarray(bq, dtype=np.float32)
    bv = np.asarray(bv, dtype=np.float32)
    bo = np.asarray(bo, dtype=np.float32)

    xT = {}
    for b in range(B):
        xT[("q", b)] = np.ascontiguousarray(q[b].T).astype(f16)
        xT[("k", b)] = np.ascontiguousarray(k[b].T).astype(f16)
        xT[("v", b)] = np.ascontiguousarray(v[b].T).astype(bf)

    wqT = [
        np.ascontiguousarray(Wq[g * HG : (g + 1) * HG, :].T).astype(f16)
        for g in range(2)
    ]
    wkT = [
        np.ascontiguousarray(Wk[g * HG : (g + 1) * HG, :].T).astype(f16)
        for g in range(2)
    ]
    wvT = [
        np.ascontiguousarray(Wv[g * HG : (g + 1) * HG, :].T).astype(bf)
        for g in range(2)
    ]
    woT = [
        np.ascontiguousarray(Wo[:, g * HG : (g + 1) * HG].T).astype(bf)
        for g in range(2)
    ]
    bqg = [np.ascontiguousarray(bq[g * HG : (g + 1) * HG]) for g in range(2)]

    in_maps = []
    for core in range(NCORES):
        b, g = core // 2, core % 2
        in_maps.append(
            {
                "xq": xT[("q", b)],
                "xk": xT[("k", b)],
                "xv": xT[("v", b)],
                "wq": wqT[g],
                "wk": wkT[g],
                "wv": wvT[g],
                "wo": woT[g],
                "bq": bqg[g],
            }
        )

    res = run_bass_kernel_spmd(nc, in_maps, core_ids=list(range(NCORES)))
    LAST_RESULT = res

    # bv never enters the device kernel: sum_k P_k bv = r * bv, so after
    # normalization it contributes exactly bv per token -> Wo @ bv on host.
    const = bo + Wo @ bv
    outp = np.empty((B, L, D), dtype=np.float32)
    for b in range(B):
        acc = res.results[2 * b]["out"].T + res.results[2 * b + 1]["out"].T
        outp[b] = acc + const
    return outp


# revision 15
# speedup vs baseline: 1.0369x; 1.0369x over previous
"""Multi-head attention TRN2 kernel: 8 cores = 4 batch x 2 head-groups.

Per core (b = core//2, g = core%2): attention for batch b, heads [8g, 8g+8),
producing the transposed partial output projection. Host sums the two
head-group partials per batch and adds (bo + Wo @ bv) once.

Precision: fp16 for the Q/K chain (x, W, Q^T, K^T, QK matmul — 10-bit
mantissa keeps score error small on std-8 scores), bf16 for the V/P/C/Wo
chain (range needed for exp(s-45) ~ 1e-20..1e16). PSUM f32.

Layouts (per core, host-prepped):
  xq/xk : x^T   [1024 d, 2048 t] f16
  xv    : x^T   [1024 d, 2048 t] bf16
  wq/wk : W_g^T [1024 d, 512 j]  f16
  wv    : W_g^T [1024 d, 512 j]  bf16
  wo    : Wo_g^T [512 c, 1024 j] bf16
  out   : OUT^T partial [1024 j, 2048 t] f32

The scalar engine's exp (256 ACTIVATEs of [128,1024], ~18.5us per
(pair,q-chunk)) is the co-critical resource next to the PE, so the schedule
is built around starting it early and never starving it: K proj, Q proj
(t-chunk 0 only needed), then QK blocks for q-chunk 0 begin immediately
while V proj and the remaining Q proj chunks fill the PE between them.
pt (exp output) is triple-buffered so QK(n) never waits on AV(n-2)'s
reads. Steady state runs a depth-3 software pipeline av(n-3); qk(n);
with out-proj for q-chunk q slotted after qk(4q+6) to cover the last
pair's normalization latency. Weight tiles for wk/wq/wv rotate through one
double-buffered slot (their lifetimes are disjoint); ct rotates per
q-chunk; x chunks load in half-d-blocks, triple buffered.
"""

import numpy as np
import ml_dtypes

D = 1024          # d_model
L = 2048          # sequence length
B = 4             # batch
HG = 512          # head-group width (8 heads x 64)
NCORES = 8
EXP_BIAS = -45.0  # softmax shift: exp(s-45); cancels in normalization

NT = 4            # token chunks of 512
TC = L // NT      # 512
NDB = D // 128    # 8 d-model blocks
NP = 4            # head pairs per group
NKB = L // 128    # 16 key blocks

_COMPILED = None
LAST_RESULT = None


def _build():
    import concourse.bacc as bacc
    import concourse.mybir as mybir
    import concourse.tile as tile

    f32 = mybir.dt.float32
    f16 = mybir.dt.float16
    bf16 = mybir.dt.bfloat16
    EXP = mybir.ActivationFunctionType.Exp
    MUL = mybir.AluOpType.mult

    nc = bacc.Bacc()

    xq = nc.declare_dram_parameter("xq", [D, L], f16, isOutput=False)
    xk = nc.declare_dram_parameter("xk", [D, L], f16, isOutput=False)
    xv = nc.declare_dram_parameter("xv", [D, L], bf16, isOutput=False)
    wq = nc.declare_dram_parameter("wq", [D, HG], f16, isOutput=False)
    wk = nc.declare_dram_parameter("wk", [D, HG], f16, isOutput=False)
    wv = nc.declare_dram_parameter("wv", [D, HG], bf16, isOutput=False)
    wo = nc.declare_dram_parameter("wo", [HG, D], bf16, isOutput=False)
    bq = nc.declare_dram_parameter("bq", [HG], f32, isOutput=False)
    out = nc.declare_dram_parameter("out", [D, L], f32, isOutput=True)

    out_v = out.rearrange("(ob p) (n t) -> ob p n t", p=128, t=TC)

    with tile.TileContext(nc) as tc:
        with tc.tile_pool(name="res", bufs=1) as res, tc.tile_pool(
            name="pa", bufs=1
        ) as pa, tc.tile_pool(name="psum", bufs=1, space="PSUM") as psum:
            # ---- resident tiles ----
            kt_sb = res.tile([128, NP, L], f16)
            qt_sb = res.tile([128, NP, L], f16)
            wo_sb = res.tile([128, NP, D], bf16)
            bq_sb = res.tile([128, NP], f32)
            bias_exp = res.tile([128, 1], f32)
            nc.vector.memset(bias_exp[:], EXP_BIAS)

            # V stationary: per (kb, pair): [V_even(64) | 1 | V_odd(64) | 1]
            # even AV uses cols 0:65, odd cols 65:130; both land at PSUM
            # partitions 0:65 with the rowsum at partition 64.
            v_sb = res.tile([128, NKB, NP, 130], bf16)
            nc.vector.memset(v_sb[:, :, :, 64:65], 1.0)
            nc.vector.memset(v_sb[:, :, :, 129:130], 1.0)

            def wtile(dt):
                return pa.tile([128, NDB, HG], dt, name="w", tag="w", bufs=2)

            def load_x_half(src, t, h, dt):
                xt = pa.tile([128, NDB // 2, TC], dt, name="xt", tag="xt", bufs=3)
                nc.sync.dma_start(
                    out=xt[:],
                    in_=src.rearrange("(db p) (n t) -> p db n t", p=128, t=TC)[
                        :, 4 * h : 4 * h + 4, t
                    ],
                )
                return xt

            def emit_kqproj(src, w_sb, t, is_q):
                xh = [load_x_half(src, t, h, f16) for h in range(2)]
                for jb in range(NP):
                    ps = psum.tile([128, TC], f32, name="acc", tag="accu", bufs=2)
                    for db in range(NDB):
                        nc.tensor.matmul(
                            ps[:],
                            w_sb[:, db, jb * 128 : (jb + 1) * 128],
                            xh[db // 4][:, db % 4, :],
                            start=(db == 0),
                            stop=(db == NDB - 1),
                        )
                    if is_q:
                        nc.vector.tensor_scalar_add(
                            qt_sb[:, jb, t * TC : (t + 1) * TC],
                            ps[:],
                            bq_sb[:, jb : jb + 1],
                        )
                    else:
                        nc.vector.tensor_copy(
                            kt_sb[:, jb, t * TC : (t + 1) * TC], ps[:]
                        )

            def emit_vproj(wv_sb, t):
                xh = [load_x_half(xv, t, h, bf16) for h in range(2)]
                for tb in range(4):
                    kb = t * 4 + tb
                    ps = psum.tile(
                        [128, NP, 128], f32, name="acc", tag="accu", bufs=2
                    )
                    for db in range(NDB):
                        nc.tensor.matmul(
                            ps[:],
                            xh[db // 4][:, db % 4, tb * 128 : (tb + 1) * 128],
                            wv_sb[:, db, :],
                            start=(db == 0),
                            stop=(db == NDB - 1),
                        )
                    nc.vector.tensor_copy(v_sb[:, kb, :, 0:64], ps[:, :, 0:64])
                    nc.vector.tensor_copy(v_sb[:, kb, :, 65:129], ps[:, :, 64:128])

            def new_pt():
                return pa.tile([128, NKB, 2, TC], bf16, name="pt", tag="pt", bufs=3)

            def emit_qk_kbs(p, q, pt, kbs):
                qsl = slice(q * TC, (q + 1) * TC)
                for kb in kbs:
                    ps_s = psum.tile(
                        [128, 2, TC], f32, name="ps_s", tag="ps_s", bufs=2
                    )
                    nc.tensor.matmul(
                        ps_s[:, 0, :],
                        kt_sb[0:64, p, kb * 128 : (kb + 1) * 128],
                        qt_sb[0:64, p, qsl],
                        start=True,
                        stop=True,
                    )
                    nc.tensor.matmul(
                        ps_s[:, 1, :],
                        kt_sb[64:128, p, kb * 128 : (kb + 1) * 128],
                        qt_sb[64:128, p, qsl],
                        start=True,
                        stop=True,
                    )
                    nc.scalar.activation(
                        pt[:, kb, :, :], ps_s[:], EXP, bias=bias_exp[:], scale=1.0
                    )

            def emit_qk(p, q):
                pt = new_pt()
                emit_qk_kbs(p, q, pt, range(NKB))
                return pt

            def emit_av(p, ct, pt):
                ps_u = psum.tile([128, 2, TC], f32, name="ps_u", tag="accu", bufs=2)
                for kb in range(NKB):
                    nc.tensor.matmul(
                        ps_u[0:65, 0, :],
                        v_sb[:, kb, p, 0:65],
                        pt[:, kb, 0, :],
                        start=(kb == 0),
                        stop=(kb == NKB - 1),
                    )
                    nc.tensor.matmul(
                        ps_u[0:65, 1, :],
                        v_sb[:, kb, p, 65:130],
                        pt[:, kb, 1, :],
                        start=(kb == 0),
                        stop=(kb == NKB - 1),
                    )
                # normalize: ct = U^T * (1/r); odd head computed at 0:64 then
                # DMA-shifted to partitions 64:128
                rr = pa.tile([1, 2, TC], f32, name="rr", tag="rr", bufs=1)
                nc.vector.reciprocal(rr[:], ps_u[64:65, :, :])
                rb = pa.tile([64, 2, TC], f32, name="rb", tag="rb", bufs=1)
                nc.gpsimd.partition_broadcast(rb[:, 0, :], rr[:, 0, :], channels=64)
                nc.gpsimd.partition_broadcast(rb[:, 1, :], rr[:, 1, :], channels=64)
                nc.vector.tensor_tensor(
                    out=ct[0:64, p, :],
                    in0=ps_u[0:64, 0, :],
                    in1=rb[:, 0, :],
                    op=MUL,
                )
                ct_o = pa.tile([64, TC], bf16, name="ct_o", tag="ct_o", bufs=2)
                nc.vector.tensor_tensor(
                    out=ct_o[:], in0=ps_u[0:64, 1, :], in1=rb[:, 1, :], op=MUL
                )
                nc.sync.dma_start(out=ct[64:128, p, :], in_=ct_o[:])

            def emit_outproj(q, ct):
                for ob in range(NDB):
                    ps = psum.tile([128, TC], f32, name="po", tag="accu", bufs=2)
                    for p in range(NP):
                        nc.tensor.matmul(
                            ps[:],
                            wo_sb[:, p, ob * 128 : (ob + 1) * 128],
                            ct[:, p, :],
                            start=(p == 0),
                            stop=(p == NP - 1),
                        )
                    o_sb = pa.tile([128, TC], f32, name="o_sb", tag="o_sb", bufs=3)
                    nc.vector.tensor_copy(o_sb[:], ps[:])
                    nc.sync.dma_start(out=out_v[ob, :, q], in_=o_sb[:])

            # ---- schedule ----
            # qk(0,0) is burst-interleaved with the K projection at kb
            # granularity (kb 4t..4t+3 only needs K proj chunk t), so the
            # scalar engine's exp stream starts ~19us into the kernel.
            wk_sb = wtile(f16)
            nc.sync.dma_start(
                out=wk_sb[:], in_=wk.rearrange("(db p) j -> p db j", p=128)
            )
            emit_kqproj(xk, wk_sb, 0, False)
            wq_sb = wtile(f16)
            nc.sync.dma_start(
                out=wq_sb[:], in_=wq.rearrange("(db p) j -> p db j", p=128)
            )
            nc.sync.dma_start(out=bq_sb[:], in_=bq.rearrange("(jb p) -> p jb", p=128))
            emit_kqproj(xq, wq_sb, 0, True)

            pts = {}
            cts = {}

            def new_ct():
                return pa.tile([128, NP, TC], bf16, name="ct", tag="ct", bufs=2)

            pts[0] = new_pt()
            emit_qk_kbs(0, 0, pts[0], range(0, 4))
            for t in range(1, NT):
                emit_kqproj(xk, wk_sb, t, False)
                emit_qk_kbs(0, 0, pts[0], range(4 * t, 4 * t + 4))
            pts[1] = emit_qk(1, 0)
            wv_sb = wtile(bf16)
            nc.sync.dma_start(
                out=wv_sb[:], in_=wv.rearrange("(db p) j -> p db j", p=128)
            )
            emit_vproj(wv_sb, 0)
            emit_vproj(wv_sb, 1)
            pts[2] = emit_qk(2, 0)
            emit_vproj(wv_sb, 2)
            emit_vproj(wv_sb, 3)
            nc.sync.dma_start(
                out=wo_sb[:], in_=wo.rearrange("(cb p) j -> p cb j", p=128)
            )

            # depth-3 pipeline: av(n-3); qk(n). Q proj chunks 1-3 ride along
            # as PE fillers right before the q-chunk that first needs them.
            for n in range(3, 16):
                if n in (4, 5, 6):
                    emit_kqproj(xq, wq_sb, n - 3, True)
                m = n - 3
                if m % 4 == 0:
                    cts[m // 4] = new_ct()
                emit_av(m % 4, cts[m // 4], pts.pop(m))
                pts[n] = emit_qk(n % 4, n // 4)
                if n % 4 == 2 and n > 3:
                    emit_outproj(n // 4 - 1, cts[n // 4 - 1])
            for m in (13, 14, 15):
                if m % 4 == 0:
                    cts[m // 4] = new_ct()
                emit_av(m % 4, cts[m // 4], pts.pop(m))
            emit_outproj(3, cts[3])

    nc.compile()
    return nc


def _get_compiled():
    global _COMPILED
    if _COMPILED is None:
        _COMPILED = _build()
    return _COMPILED


def kernel(q, k, v, Wq, bq, Wk, bk, Wv, bv, Wo, bo):
    global LAST_RESULT
    from concourse.bass_utils import run_bass_kernel_spmd

    nc = _get_compiled()

    f16 = np.float16
    bf = ml_dtypes.bfloat16
    q = np.asarray(q, dtype=np.float32)
    k = np.asarray(k, dtype=np.float32)
    v = np.asarray(v, dtype=np.float32)
    Wq = np.asarray(Wq, dtype=np.float32)
    Wk = np.asarray(Wk, dtype=np.float32)
    Wv = np.asarray(Wv, dtype=np.float32)
    Wo = np.asarray(Wo, dtype=np.float32)
    bq = np.asarray(bq, dtype=np.float32)
    bv = np.asarray(bv, dtype=np.float32)
    bo = np.asarray(bo, dtype=np.float32)

    xT = {}
    for b in range(B):
        xT[("q", b)] = np.ascontiguousarray(q[b].T).astype(f16)
        xT[("k", b)] = np.ascontiguousarray(k[b].T).astype(f16)
        xT[("v", b)] = np.ascontiguousarray(v[b].T).astype(bf)

    wqT = [
        np.ascontiguousarray(Wq[g * HG : (g + 1) * HG, :].T).astype(f16)
        for g in range(2)
    ]
    wkT = [
        np.ascontiguousarray(Wk[g * HG : (g + 1) * HG, :].T).astype(f16)
        for g in range(2)
    ]
    wvT = [
        np.ascontiguousarray(Wv[g * HG : (g + 1) * HG, :].T).astype(bf)
        for g in range(2)
    ]
    woT = [
        np.ascontiguousarray(Wo[:, g * HG : (g + 1) * HG].T).astype(bf)
        for g in range(2)
    ]
    bqg = [np.ascontiguousarray(bq[g * HG : (g + 1) * HG]) for g in range(2)]

    in_maps = []
    for core in range(NCORES):
        b, g = core // 2, core % 2
        in_maps.append(
            {
                "xq": xT[("q", b)],
                "xk": xT[("k", b)],
                "xv": xT[("v", b)],
                "wq": wqT[g],
                "wk": wkT[g],
                "wv": wvT[g],
                "wo": woT[g],
                "bq": bqg[g],
            }
        )

    res = run_bass_kernel_spmd(nc, in_maps, core_ids=list(range(NCORES)))
    LAST_RESULT = res

    # bv never enters the device kernel: sum_k P_k bv = r * bv, so after
    # normalization it contributes exactly bv per token -> Wo @ bv on host.
    const = bo + Wo @ bv
    outp = np.empty((B, L, D), dtype=np.float32)
    for b in range(B):
        acc = res.results[2 * b]["out"].T + res.results[2 * b + 1]["out"].T
        outp[b] = acc + const
    return outp


# revision 24
# speedup vs baseline: 1.1117x; 1.0722x over previous
"""Multi-head attention TRN2 kernel: 8 cores = 4 batch x 2 head-groups.

Per core (b = core//2, g = core%2): attention for batch b, heads [8g, 8g+8),
producing the transposed partial output projection. Host sums the two
head-group partials per batch and adds (bo + Wo @ bv) once.

Precision: fp16 for the Q/K chain (x, W, Q^T, K^T, QK matmul — 10-bit
mantissa keeps score error small on std-8 scores), bf16 for the V/P/C/Wo
chain (range needed for exp(s-45) ~ 1e-20..1e16). PSUM f32.

Layouts (per core, host-prepped):
  xq/xk : x^T   [1024 d, 2048 t] f16
  xv    : x^T   [1024 d, 2048 t] bf16
  wq/wk : W_g^T [1024 d, 512 j]  f16
  wv    : W_g^T [1024 d, 512 j]  bf16
  wo    : Wo_g^T [512 c, 1024 j] bf16
  out   : OUT^T partial [1024 j, 2048 t] f32

The scalar engine's exp (256 ACTIVATEs of [128,1024], ~18.5us per
(pair,q-chunk)) is the co-critical resource next to the PE, so the schedule
is built around starting it early and never starving it: K proj, Q proj
(t-chunk 0 only needed), then QK blocks for q-chunk 0 begin immediately
while V proj and the remaining Q proj chunks fill the PE between them.
pt (exp output) is triple-buffered so QK(n) never waits on AV(n-2)'s
reads. Steady state runs a depth-3 software pipeline av(n-3); qk(n);
with out-proj for q-chunk q slotted after qk(4q+6) to cover the last
pair's normalization latency. Weight tiles for wk/wq/wv rotate through one
double-buffered slot (their lifetimes are disjoint); ct rotates per
q-chunk; x chunks load in half-d-blocks, triple buffered.
"""

import numpy as np
import ml_dtypes

D = 1024          # d_model
L = 2048          # sequence length
B = 4             # batch
HG = 512          # head-group width (8 heads x 64)
NCORES = 8
EXP_BIAS = -45.0  # softmax shift: exp(s-45); cancels in normalization

NT = 4            # token chunks of 512
TC = L // NT      # 512
NDB = D // 128    # 8 d-model blocks
NP = 4            # head pairs per group
NKB = L // 128    # 16 key blocks

_COMPILED = None
LAST_RESULT = None


def _build():
    import concourse.bacc as bacc
    import concourse.mybir as mybir
    import concourse.tile as tile

    f32 = mybir.dt.float32
    f16 = mybir.dt.float16
    bf16 = mybir.dt.bfloat16
    EXP = mybir.ActivationFunctionType.Exp
    MUL = mybir.AluOpType.mult

    nc = bacc.Bacc()

    xq = nc.declare_dram_parameter("xq", [D, L], f16, isOutput=False)
    xk = nc.declare_dram_parameter("xk", [D, L], f16, isOutput=False)
    xv = nc.declare_dram_parameter("xv", [D, L], bf16, isOutput=False)
    wq = nc.declare_dram_parameter("wq", [D, HG], f16, isOutput=False)
    wk = nc.declare_dram_parameter("wk", [D, HG], f16, isOutput=False)
    wv = nc.declare_dram_parameter("wv", [D, HG], bf16, isOutput=False)
    wo = nc.declare_dram_parameter("wo", [HG, D], bf16, isOutput=False)
    bq = nc.declare_dram_parameter("bq", [HG], f32, isOutput=False)
    out = nc.declare_dram_parameter("out", [D, L], f32, isOutput=True)

    out_v = out.rearrange("(ob p) (n t) -> ob p n t", p=128, t=TC)

    with tile.TileContext(nc) as tc:
        with tc.tile_pool(name="res", bufs=1) as res, tc.tile_pool(
            name="pa", bufs=1
        ) as pa, tc.tile_pool(name="psum", bufs=1, space="PSUM") as psum:
            # ---- resident tiles ----
            kt_sb = res.tile([128, NP, L], f16)
            qt_sb = res.tile([128, NP, L], f16)
            wo_sb = res.tile([128, NP, D], bf16)
            bq_sb = res.tile([128, NP], f32)
            bias_exp = res.tile([128, 1], f32)
            nc.vector.memset(bias_exp[:], EXP_BIAS)

            # V stationary: per (kb, pair): [V_even(64) | 1 | V_odd(64) | 1]
            # even AV uses cols 0:65, odd cols 65:130; both land at PSUM
            # partitions 0:65 with the rowsum at partition 64.
            v_sb = res.tile([128, NKB, NP, 130], bf16)
            nc.vector.memset(v_sb[:, :, :, 64:65], 1.0)
            nc.vector.memset(v_sb[:, :, :, 129:130], 1.0)

            def wtile(dt):
                return pa.tile([128, NDB, HG], dt, name="w", tag="w", bufs=2)

            def load_x_half(src, t, h, dt, eng=None):
                xt = pa.tile([128, NDB // 2, TC], dt, name="xt", tag="xt", bufs=3)
                (eng or nc.sync).dma_start(
                    out=xt[:],
                    in_=src.rearrange("(db p) (n t) -> p db n t", p=128, t=TC)[
                        :, 4 * h : 4 * h + 4, t
                    ],
                )
                return xt

            def emit_kqproj(src, w_sb, t, is_q, x_engs=(None, None)):
                xh = [load_x_half(src, t, h, f16, x_engs[h]) for h in range(2)]
                for jb in range(NP):
                    ps = psum.tile([128, TC], f32, name="acc", tag="accu", bufs=2)
                    for db in range(NDB):
                        nc.tensor.matmul(
                            ps[:],
                            w_sb[:, db, jb * 128 : (jb + 1) * 128],
                            xh[db // 4][:, db % 4, :],
                            start=(db == 0),
                            stop=(db == NDB - 1),
                        )
                    if is_q:
                        nc.vector.tensor_scalar_add(
                            qt_sb[:, jb, t * TC : (t + 1) * TC],
                            ps[:],
                            bq_sb[:, jb : jb + 1],
                        )
                    else:
                        nc.vector.tensor_copy(
                            kt_sb[:, jb, t * TC : (t + 1) * TC], ps[:]
                        )

            def emit_vproj(wv_sb, t):
                xh = [load_x_half(xv, t, h, bf16) for h in range(2)]
                for tb in range(4):
                    kb = t * 4 + tb
                    ps = psum.tile(
                        [128, NP, 128], f32, name="acc", tag="accu", bufs=2
                    )
                    for db in range(NDB):
                        nc.tensor.matmul(
                            ps[:],
                            xh[db // 4][:, db % 4, tb * 128 : (tb + 1) * 128],
                            wv_sb[:, db, :],
                            start=(db == 0),
                            stop=(db == NDB - 1),
                        )
                    nc.vector.tensor_copy(v_sb[:, kb, :, 0:64], ps[:, :, 0:64])
                    nc.vector.tensor_copy(v_sb[:, kb, :, 65:129], ps[:, :, 64:128])

            def new_pt():
                return pa.tile([128, NKB, 2, TC], bf16, name="pt", tag="pt", bufs=3)

            def emit_qk_kbs(p, q, pt, kbs):
                qsl = slice(q * TC, (q + 1) * TC)
                for kb in kbs:
                    ps_s = psum.tile(
                        [128, 2, TC], f32, name="ps_s", tag="ps_s", bufs=2
                    )
                    nc.tensor.matmul(
                        ps_s[:, 0, :],
                        kt_sb[0:64, p, kb * 128 : (kb + 1) * 128],
                        qt_sb[0:64, p, qsl],
                        start=True,
                        stop=True,
                    )
                    nc.tensor.matmul(
                        ps_s[:, 1, :],
                        kt_sb[64:128, p, kb * 128 : (kb + 1) * 128],
                        qt_sb[64:128, p, qsl],
                        start=True,
                        stop=True,
                    )
                    nc.scalar.activation(
                        pt[:, kb, :, :], ps_s[:], EXP, bias=bias_exp[:], scale=1.0
                    )

            def emit_qk(p, q):
                pt = new_pt()
                emit_qk_kbs(p, q, pt, range(NKB))
                return pt

            def emit_av(p, ct, pt):
                ps_u = psum.tile([128, 2, TC], f32, name="ps_u", tag="accu", bufs=2)
                for kb in range(NKB):
                    nc.tensor.matmul(
                        ps_u[0:65, 0, :],
                        v_sb[:, kb, p, 0:65],
                        pt[:, kb, 0, :],
                        start=(kb == 0),
                        stop=(kb == NKB - 1),
                    )
                    nc.tensor.matmul(
                        ps_u[0:65, 1, :],
                        v_sb[:, kb, p, 65:130],
                        pt[:, kb, 1, :],
                        start=(kb == 0),
                        stop=(kb == NKB - 1),
                    )
                # normalize: ct = U^T * (1/r); odd head computed at 0:64 then
                # DMA-shifted to partitions 64:128
                # custom-DVE ops misread PSUM: bounce the rowsum row to SBUF
                # first, then approx-recip (regular reciprocal is ~6.5us/row)
                rr_raw = pa.tile([1, 2, TC], f32, name="rr_raw", tag="rrw", bufs=1)
                nc.vector.tensor_copy(rr_raw[:], ps_u[64:65, :, :])
                rr = pa.tile([1, 2, TC], f32, name="rr", tag="rr", bufs=1)
                nc.vector.reciprocal_approx_fast(rr[:], rr_raw[:])
                rb = pa.tile([64, 2, TC], f32, name="rb", tag="rb", bufs=1)
                nc.gpsimd.partition_broadcast(rb[:, 0, :], rr[:, 0, :], channels=64)
                nc.gpsimd.partition_broadcast(rb[:, 1, :], rr[:, 1, :], channels=64)
                nc.vector.tensor_tensor(
                    out=ct[0:64, p, :],
                    in0=ps_u[0:64, 0, :],
                    in1=rb[:, 0, :],
                    op=MUL,
                )
                ct_o = pa.tile([64, TC], bf16, name="ct_o", tag="ct_o", bufs=1)
                nc.vector.tensor_tensor(
                    out=ct_o[:], in0=ps_u[0:64, 1, :], in1=rb[:, 1, :], op=MUL
                )
                nc.sync.dma_start(out=ct[64:128, p, :], in_=ct_o[:])

            def emit_outproj(q, ct):
                for ob in range(NDB):
                    ps = psum.tile([128, TC], f32, name="po", tag="accu", bufs=2)
                    for p in range(NP):
                        nc.tensor.matmul(
                            ps[:],
                            wo_sb[:, p, ob * 128 : (ob + 1) * 128],
                            ct[:, p, :],
                            start=(p == 0),
                            stop=(p == NP - 1),
                        )
                    o_sb = pa.tile([128, TC], f32, name="o_sb", tag="o_sb", bufs=2)
                    nc.vector.tensor_copy(o_sb[:], ps[:])
                    nc.sync.dma_start(out=out_v[ob, :, q], in_=o_sb[:])

            # ---- schedule ----
            # qk(0,0) is burst-interleaved with the K projection at kb
            # granularity (kb 4t..4t+3 only needs K proj chunk t), so the
            # scalar engine's exp stream starts ~19us into the kernel.
            # startup DMAs fan out across the four engines' DGE queues so the
            # first projection matmul isn't gated on one serial queue
            engs = [nc.sync, nc.gpsimd, nc.scalar, nc.sync]

            def load_w_split(w_sb, src):
                wr = src.rearrange("(db p) j -> p db j", p=128)
                for jq in range(4):
                    engs[jq].dma_start(
                        out=w_sb[:, :, jq * 128 : (jq + 1) * 128],
                        in_=wr[:, :, jq * 128 : (jq + 1) * 128],
                    )

            wk_sb = wtile(f16)
            load_w_split(wk_sb, wk)
            emit_kqproj(xk, wk_sb, 0, False, x_engs=(nc.gpsimd, nc.scalar))
            wq_sb = wtile(f16)
            load_w_split(wq_sb, wq)
            nc.sync.dma_start(out=bq_sb[:], in_=bq.rearrange("(jb p) -> p jb", p=128))
            emit_kqproj(xq, wq_sb, 0, True, x_engs=(nc.gpsimd, nc.scalar))

            pts = {}
            cts = {}

            def new_ct():
                return pa.tile([128, NP, TC], bf16, name="ct", tag="ct", bufs=2)

            pts[0] = new_pt()
            emit_qk_kbs(0, 0, pts[0], range(0, 4))
            for t in range(1, NT):
                emit_kqproj(xk, wk_sb, t, False)
                emit_qk_kbs(0, 0, pts[0], range(4 * t, 4 * t + 4))
            pts[1] = emit_qk(1, 0)
            wv_sb = wtile(bf16)
            nc.sync.dma_start(
                out=wv_sb[:], in_=wv.rearrange("(db p) j -> p db j", p=128)
            )
            emit_vproj(wv_sb, 0)
            emit_vproj(wv_sb, 1)
            pts[2] = emit_qk(2, 0)
            emit_vproj(wv_sb, 2)
            emit_vproj(wv_sb, 3)
            nc.sync.dma_start(
                out=wo_sb[:], in_=wo.rearrange("(cb p) j -> p cb j", p=128)
            )

            # depth-3 pipeline: av(n-3); qk(n). Q proj chunks 1-3 ride along
            # as PE fillers right before the q-chunk that first needs them.
            for n in range(3, 16):
                if n in (4, 5, 6):
                    emit_kqproj(xq, wq_sb, n - 3, True)
                m = n - 3
                if m % 4 == 0:
                    cts[m // 4] = new_ct()
                emit_av(m % 4, cts[m // 4], pts.pop(m))
                pts[n] = emit_qk(n % 4, n // 4)
                if n % 4 == 2 and n > 3:
                    emit_outproj(n // 4 - 1, cts[n // 4 - 1])
            for m in (13, 14, 15):
                if m % 4 == 0:
                    cts[m // 4] = new_ct()
                emit_av(m % 4, cts[m // 4], pts.pop(m))
            emit_outproj(3, cts[3])

    nc.compile()
    return nc


def _get_compiled():
    global _COMPILED
    if _COMPILED is None:
        _COMPILED = _build()
    return _COMPILED


def kernel(q, k, v, Wq, bq, Wk, bk, Wv, bv, Wo, bo):
    global LAST_RESULT
    from concourse.bass_utils import run_bass_kernel_spmd

    nc = _get_compiled()

    f16 = np.float16
    bf = ml_dtypes.bfloat16
    q = np.asarray(q, dtype=np.float32)
    k = np.asarray(k, dtype=np.float32)
    v = np.asarray(v, dtype=np.float32)
    Wq = np.asarray(Wq, dtype=np.float32)
    Wk = np.asarray(Wk, dtype=np.float32)
    Wv = np.asarray(Wv, dtype=np.float32)
    Wo = np.asarray(Wo, dtype=np.float32)
    bq = np.asarray(bq, dtype=np.float32)
    bv = np.asarray(bv, dtype=np.float32)
    bo = np.asarray(bo, dtype=np.float32)

    xT = {}
    for b in range(B):
        xT[("q", b)] = np.ascontiguousarray(q[b].T).astype(f16)
        xT[("k", b)] = np.ascontiguousarray(k[b].T).astype(f16)
        xT[("v", b)] = np.ascontiguousarray(v[b].T).astype(bf)

    wqT = [
        np.ascontiguousarray(Wq[g * HG : (g + 1) * HG, :].T).astype(f16)
        for g in range(2)
    ]
    wkT = [
        np.ascontiguousarray(Wk[g * HG : (g + 1) * HG, :].T).astype(f16)
        for g in range(2)
    ]
    wvT = [
        np.ascontiguousarray(Wv[g * HG : (g + 1) * HG, :].T).astype(bf)
        for g in range(2)
    ]
    woT = [
        np.ascontiguousarray(Wo[:, g * HG : (g + 1) * HG].T).astype(bf)
        for g in range(2)
    ]
    bqg = [np.ascontiguousarray(bq[g * HG : (g + 1) * HG]) for g in range(2)]

    in_maps = []
    for core in range(NCORES):
        b, g = core // 2, core % 2
        in_maps.append(
            {
                "xq": xT[("q", b)],
                "xk": xT[("k", b)],
                "xv": xT[("v", b)],
                "wq": wqT[g],
                "wk": wkT[g],
                "wv": wvT[g],
                "wo": woT[g],
                "bq": bqg[g],
            }
        )

    res = run_bass_kernel_spmd(nc, in_maps, core_ids=list(range(NCORES)))
    LAST_RESULT = res

    # bv never enters the device kernel: sum_k P_k bv = r * bv, so after
    # normalization it contributes exactly bv per token -> Wo @ bv on host.
    const = bo + Wo @ bv
    outp = np.empty((B, L, D), dtype=np.float32)
    for b in range(B):
        acc = res.results[2 * b]["out"].T + res.results[2 * b + 1]["out"].T
        outp[b] = acc + const
    return outp


# revision 29
# speedup vs baseline: 1.1237x; 1.0108x over previous
"""Multi-head attention TRN2 kernel: 8 cores = 4 batch x 2 head-groups.

Per core (b = core//2, g = core%2): attention for batch b, heads [8g, 8g+8),
producing the transposed partial output projection. Host sums the two
head-group partials per batch and adds (bo + Wo @ bv) once.

Precision: fp16 for the Q/K chain (x, W, Q^T, K^T, QK matmul — 10-bit
mantissa keeps score error small on std-8 scores), bf16 for the V/P/C/Wo
chain (range needed for exp(s-45) ~ 1e-20..1e16). PSUM f32.

Layouts (per core, host-prepped):
  xq/xk : x^T   [1024 d, 2048 t] f16
  xv    : x^T   [1024 d, 2048 t] bf16
  wq/wk : W_g^T [1024 d, 512 j]  f16
  wv    : W_g^T [1024 d, 512 j]  bf16
  wo    : Wo_g^T [512 c, 1024 j] bf16
  out   : OUT^T partial [1024 j, 2048 t] f32

The scalar engine's exp (256 ACTIVATEs of [128,1024], ~18.5us per
(pair,q-chunk)) is the co-critical resource next to the PE, so the schedule
is built around starting it early and never starving it: K proj, Q proj
(t-chunk 0 only needed), then QK blocks for q-chunk 0 begin immediately
while V proj and the remaining Q proj chunks fill the PE between them.
pt (exp output) is triple-buffered so QK(n) never waits on AV(n-2)'s
reads. Steady state runs a depth-3 software pipeline av(n-3); qk(n);
with out-proj for q-chunk q slotted after qk(4q+6) to cover the last
pair's normalization latency. Weight tiles for wk/wq/wv rotate through one
double-buffered slot (their lifetimes are disjoint); ct rotates per
q-chunk; x chunks load in half-d-blocks, triple buffered.
"""

import numpy as np
import ml_dtypes

D = 1024          # d_model
L = 2048          # sequence length
B = 4             # batch
HG = 512          # head-group width (8 heads x 64)
NCORES = 8
EXP_BIAS = -45.0  # softmax shift: exp(s-45); cancels in normalization

NT = 4            # token chunks of 512
TC = L // NT      # 512
NDB = D // 128    # 8 d-model blocks
NP = 4            # head pairs per group
NKB = L // 128    # 16 key blocks

_COMPILED = None
LAST_RESULT = None


def _build():
    import concourse.bacc as bacc
    import concourse.mybir as mybir
    import concourse.tile as tile

    f32 = mybir.dt.float32
    f16 = mybir.dt.float16
    bf16 = mybir.dt.bfloat16
    EXP = mybir.ActivationFunctionType.Exp
    MUL = mybir.AluOpType.mult

    nc = bacc.Bacc()

    xq = nc.declare_dram_parameter("xq", [D, L], f16, isOutput=False)
    xk = nc.declare_dram_parameter("xk", [D, L], f16, isOutput=False)
    xv = nc.declare_dram_parameter("xv", [D, L], bf16, isOutput=False)
    wq = nc.declare_dram_parameter("wq", [D, HG], f16, isOutput=False)
    wk = nc.declare_dram_parameter("wk", [D, HG], f16, isOutput=False)
    wv = nc.declare_dram_parameter("wv", [D, HG], bf16, isOutput=False)
    wo = nc.declare_dram_parameter("wo", [HG, D], bf16, isOutput=False)
    bq = nc.declare_dram_parameter("bq", [HG], f32, isOutput=False)
    out = nc.declare_dram_parameter("out", [D, L], f32, isOutput=True)

    out_v = out.rearrange("(ob p) (n t) -> ob p n t", p=128, t=TC)

    with tile.TileContext(nc) as tc:
        with tc.tile_pool(name="res", bufs=1) as res, tc.tile_pool(
            name="pa", bufs=1
        ) as pa, tc.tile_pool(name="psum", bufs=1, space="PSUM") as psum:
            # ---- resident tiles ----
            kt_sb = res.tile([128, NP, L], f16)
            qt_sb = res.tile([128, NP, L], f16)
            wo_sb = res.tile([128, NP, D], bf16)
            bq_sb = res.tile([128, NP], f32)
            bias_exp = res.tile([128, 1], f32)
            nc.vector.memset(bias_exp[:], EXP_BIAS)

            # V stationary: per (kb, pair): [V_even(64) | 1 | V_odd(64) | 1]
            # even AV uses cols 0:65, odd cols 65:130; both land at PSUM
            # partitions 0:65 with the rowsum at partition 64.
            v_sb = res.tile([128, NKB, NP, 130], bf16)
            nc.vector.memset(v_sb[:, :, :, 64:65], 1.0)
            nc.vector.memset(v_sb[:, :, :, 129:130], 1.0)

            def wtile(dt):
                return pa.tile([128, NDB, HG], dt, name="w", tag="w", bufs=2)

            _xeng = [0]

            def load_x_half(src, t, h, dt, eng=None):
                if eng is None:
                    eng = engs[_xeng[0] % 3]
                    _xeng[0] += 1
                xt = pa.tile([128, NDB // 2, TC], dt, name="xt", tag="xt", bufs=3)
                eng.dma_start(
                    out=xt[:],
                    in_=src.rearrange("(db p) (n t) -> p db n t", p=128, t=TC)[
                        :, 4 * h : 4 * h + 4, t
                    ],
                )
                return xt

            def emit_kqproj(src, w_sb, t, is_q, x_engs=(None, None)):
                xh = [load_x_half(src, t, h, f16, x_engs[h]) for h in range(2)]
                for jb in range(NP):
                    ps = psum.tile([128, TC], f32, name="acc", tag="accu", bufs=2)
                    for db in range(NDB):
                        nc.tensor.matmul(
                            ps[:],
                            w_sb[:, db, jb * 128 : (jb + 1) * 128],
                            xh[db // 4][:, db % 4, :],
                            start=(db == 0),
                            stop=(db == NDB - 1),
                        )
                    if is_q:
                        nc.vector.tensor_scalar_add(
                            qt_sb[:, jb, t * TC : (t + 1) * TC],
                            ps[:],
                            bq_sb[:, jb : jb + 1],
                        )
                    else:
                        nc.vector.tensor_copy(
                            kt_sb[:, jb, t * TC : (t + 1) * TC], ps[:]
                        )

            def emit_vproj(wv_sb, t):
                xh = [load_x_half(xv, t, h, bf16) for h in range(2)]
                for tb in range(4):
                    kb = t * 4 + tb
                    ps = psum.tile(
                        [128, NP, 128], f32, name="acc", tag="accu", bufs=2
                    )
                    for db in range(NDB):
                        nc.tensor.matmul(
                            ps[:],
                            xh[db // 4][:, db % 4, tb * 128 : (tb + 1) * 128],
                            wv_sb[:, db, :],
                            start=(db == 0),
                            stop=(db == NDB - 1),
                        )
                    nc.vector.tensor_copy(v_sb[:, kb, :, 0:64], ps[:, :, 0:64])
                    nc.vector.tensor_copy(v_sb[:, kb, :, 65:129], ps[:, :, 64:128])

            def new_pt():
                return pa.tile([128, NKB, 2, TC], bf16, name="pt", tag="pt", bufs=3)

            def emit_qk_kbs(p, q, pt, kbs):
                qsl = slice(q * TC, (q + 1) * TC)
                for kb in kbs:
                    ps_s = psum.tile(
                        [128, 2, TC], f32, name="ps_s", tag="ps_s", bufs=2
                    )
                    nc.tensor.matmul(
                        ps_s[:, 0, :],
                        kt_sb[0:64, p, kb * 128 : (kb + 1) * 128],
                        qt_sb[0:64, p, qsl],
                        start=True,
                        stop=True,
                    )
                    nc.tensor.matmul(
                        ps_s[:, 1, :],
                        kt_sb[64:128, p, kb * 128 : (kb + 1) * 128],
                        qt_sb[64:128, p, qsl],
                        start=True,
                        stop=True,
                    )
                    nc.scalar.activation(
                        pt[:, kb, :, :], ps_s[:], EXP, bias=bias_exp[:], scale=1.0
                    )

            def emit_qk(p, q):
                pt = new_pt()
                emit_qk_kbs(p, q, pt, range(NKB))
                return pt

            def emit_av(p, ct, pt):
                ps_u = psum.tile([128, 2, TC], f32, name="ps_u", tag="accu", bufs=2)
                for kb in range(NKB):
                    nc.tensor.matmul(
                        ps_u[0:65, 0, :],
                        v_sb[:, kb, p, 0:65],
                        pt[:, kb, 0, :],
                        start=(kb == 0),
                        stop=(kb == NKB - 1),
                    )
                    nc.tensor.matmul(
                        ps_u[0:65, 1, :],
                        v_sb[:, kb, p, 65:130],
                        pt[:, kb, 1, :],
                        start=(kb == 0),
                        stop=(kb == NKB - 1),
                    )
                # normalize: ct = U^T * (1/r); odd head computed at 0:64 then
                # DMA-shifted to partitions 64:128
                # custom-DVE ops misread PSUM: bounce the rowsum row to SBUF
                # first, then approx-recip (regular reciprocal is ~6.5us/row)
                rr_raw = pa.tile([1, 2, TC], f32, name="rr_raw", tag="rrw", bufs=1)
                nc.vector.tensor_copy(rr_raw[:], ps_u[64:65, :, :])
                rr = pa.tile([1, 2, TC], f32, name="rr", tag="rr", bufs=1)
                nc.vector.reciprocal_approx_fast(rr[:], rr_raw[:])
                rb = pa.tile([64, 2, TC], f32, name="rb", tag="rb", bufs=1)
                nc.gpsimd.partition_broadcast(rb[:, 0, :], rr[:, 0, :], channels=64)
                nc.gpsimd.partition_broadcast(rb[:, 1, :], rr[:, 1, :], channels=64)
                nc.vector.tensor_tensor(
                    out=ct[0:64, p, :],
                    in0=ps_u[0:64, 0, :],
                    in1=rb[:, 0, :],
                    op=MUL,
                )
                ct_o = pa.tile([64, TC], bf16, name="ct_o", tag="ct_o", bufs=1)
                nc.vector.tensor_tensor(
                    out=ct_o[:], in0=ps_u[0:64, 1, :], in1=rb[:, 1, :], op=MUL
                )
                nc.sync.dma_start(out=ct[64:128, p, :], in_=ct_o[:])

            def emit_outproj(q, ct):
                for ob in range(NDB):
                    ps = psum.tile([128, TC], f32, name="po", tag="accu", bufs=2)
                    for p in range(NP):
                        nc.tensor.matmul(
                            ps[:],
                            wo_sb[:, p, ob * 128 : (ob + 1) * 128],
                            ct[:, p, :],
                            start=(p == 0),
                            stop=(p == NP - 1),
                        )
                    o_sb = pa.tile([128, TC], f32, name="o_sb", tag="o_sb", bufs=3)
                    nc.vector.tensor_copy(o_sb[:], ps[:])
                    engs[ob % 3].dma_start(out=out_v[ob, :, q], in_=o_sb[:])

            # ---- schedule ----
            # qk(0,0) is burst-interleaved with the K projection at kb
            # granularity (kb 4t..4t+3 only needs K proj chunk t), so the
            # scalar engine's exp stream starts ~19us into the kernel.
            # startup DMAs are choreographed across the three DMA-capable
            # engines' queues so the first projection matmuls aren't gated on
            # one serial queue: wk quarters stream on sync (jb loop consumes
            # them in order), xk halves lead the gpsimd/scalar queues, wq
            # rides gpsimd behind xk-h0, xq splits scalar/sync.
            engs = [nc.sync, nc.gpsimd, nc.scalar, nc.sync]

            def load_w_split(w_sb, src, elist):
                wr = src.rearrange("(db p) j -> p db j", p=128)
                for jq in range(4):
                    elist[jq].dma_start(
                        out=w_sb[:, :, jq * 128 : (jq + 1) * 128],
                        in_=wr[:, :, jq * 128 : (jq + 1) * 128],
                    )

            wk_sb = wtile(f16)
            load_w_split(wk_sb, wk, [nc.sync] * 4)
            emit_kqproj(xk, wk_sb, 0, False, x_engs=(nc.gpsimd, nc.scalar))
            wq_sb = wtile(f16)
            load_w_split(wq_sb, wq, [nc.gpsimd] * 4)
            nc.scalar.dma_start(
                out=bq_sb[:], in_=bq.rearrange("(jb p) -> p jb", p=128)
            )
            emit_kqproj(xq, wq_sb, 0, True, x_engs=(nc.scalar, nc.sync))

            pts = {}
            cts = {}

            def new_ct():
                return pa.tile([128, NP, TC], bf16, name="ct", tag="ct", bufs=2)

            pts[0] = new_pt()
            emit_qk_kbs(0, 0, pts[0], range(0, 4))
            for t in range(1, NT):
                emit_kqproj(xk, wk_sb, t, False)
                emit_qk_kbs(0, 0, pts[0], range(4 * t, 4 * t + 4))
            pts[1] = emit_qk(1, 0)
            wv_sb = wtile(bf16)
            nc.sync.dma_start(
                out=wv_sb[:], in_=wv.rearrange("(db p) j -> p db j", p=128)
            )
            emit_vproj(wv_sb, 0)
            emit_vproj(wv_sb, 1)
            pts[2] = emit_qk(2, 0)
            emit_vproj(wv_sb, 2)
            emit_vproj(wv_sb, 3)
            nc.sync.dma_start(
                out=wo_sb[:], in_=wo.rearrange("(cb p) j -> p cb j", p=128)
            )

            # depth-3 pipeline: av(n-3); qk(n). Q proj chunks 1-3 ride along as
            # PE fillers, one per q-cycle (qproj(i) just before qk(4i) needs it)
            # so each 4-pair cycle is PE 68us vs ACT 74us — ACT stays pacer.
            for n in range(3, 16):
                if n in (3, 7, 11):
                    emit_kqproj(xq, wq_sb, (n - 3) // 4 + 1, True)
                m = n - 3
                if m % 4 == 0:
                    cts[m // 4] = new_ct()
                emit_av(m % 4, cts[m // 4], pts.pop(m))
                pts[n] = emit_qk(n % 4, n // 4)
                if n % 4 == 2 and n > 3:
                    emit_outproj(n // 4 - 1, cts[n // 4 - 1])
            for m in (13, 14, 15):
                if m % 4 == 0:
                    cts[m // 4] = new_ct()
                emit_av(m % 4, cts[m // 4], pts.pop(m))
            emit_outproj(3, cts[3])

    nc.compile()
    return nc


def _get_compiled():
    global _COMPILED
    if _COMPILED is None:
        _COMPILED = _build()
    return _COMPILED


def kernel(q, k, v, Wq, bq, Wk, bk, Wv, bv, Wo, bo):
    global LAST_RESULT
    from concourse.bass_utils import run_bass_kernel_spmd

    nc = _get_compiled()

    f16 = np.float16
    bf = ml_dtypes.bfloat16
    q = np.asarray(q, dtype=np.float32)
    k = np.asarray(k, dtype=np.float32)
    v = np.asarray(v, dtype=np.float32)
    Wq = np.asarray(Wq, dtype=np.float32)
    Wk = np.asarray(Wk, dtype=np.float32)
    Wv = np.asarray(Wv, dtype=np.float32)
    Wo = np.asarray(Wo, dtype=np.float32)
    bq = np.asarray(bq, dtype=np.float32)
    bv = np.asarray(bv, dtype=np.float32)
    bo = np.asarray(bo, dtype=np.float32)

    xT = {}
    for b in range(B):
        xT[("q", b)] = np.ascontiguousarray(q[b].T).astype(f16)
        xT[("k", b)] = np.ascontiguousarray(k[b].T).astype(f16)
        xT[("v", b)] = np.ascontiguousarray(v[b].T).astype(bf)

    wqT = [
        np.ascontiguousarray(Wq[g * HG : (g + 1) * HG, :].T).astype(f16)
        for g in range(2)
    ]
    wkT = [
        np.ascontiguousarray(Wk[g * HG : (g + 1) * HG, :].T).astype(f16)
        for g in range(2)
    ]
    wvT = [
        np.ascontiguousarray(Wv[g * HG : (g + 1) * HG, :].T).astype(bf)
        for g in range(2)
    ]
    woT = [
        np.ascontiguousarray(Wo[:, g * HG : (g + 1) * HG].T).astype(bf)
        for g in range(2)
    ]
    bqg = [np.ascontiguousarray(bq[g * HG : (g + 1) * HG]) for g in range(2)]

    in_maps = []
    for core in range(NCORES):
        b, g = core // 2, core % 2
        in_maps.append(
            {
                "xq": xT[("q", b)],
                "xk": xT[("k", b)],
                "xv": xT[("v", b)],
                "wq": wqT[g],
                "wk": wkT[g],
                "wv": wvT[g],
                "wo": woT[g],
                "bq": bqg[g],
            }
        )

    res = run_bass_kernel_spmd(nc, in_maps, core_ids=list(range(NCORES)))
    LAST_RESULT = res

    # bv never enters the device kernel: sum_k P_k bv = r * bv, so after
    # normalization it contributes exactly bv per token -> Wo @ bv on host.
    const = bo + Wo @ bv
    outp = np.empty((B, L, D), dtype=np.float32)
    for b in range(B):
        acc = res.results[2 * b]["out"].T + res.results[2 * b + 1]["out"].T
        outp[b] = acc + const
    return outp


# revision 34
# speedup vs baseline: 1.1256x; 1.0017x over previous
"""Multi-head attention TRN2 kernel: 8 cores = 4 batch x 2 head-groups.

Per core (b = core//2, g = core%2): attention for batch b, heads [8g, 8g+8),
producing the transposed partial output projection. Host sums the two
head-group partials per batch and adds (bo + Wo @ bv) once.

Precision: fp16 for the Q/K chain (x, W, Q^T, K^T, QK matmul — 10-bit
mantissa keeps score error small on std-8 scores), bf16 for the V/P/C/Wo
chain (range needed for exp(s-45) ~ 1e-20..1e16). PSUM f32.

Layouts (per core, host-prepped):
  xq/xk : x^T   [1024 d, 2048 t] f16
  xv    : x^T   [1024 d, 2048 t] bf16
  wq/wk : W_g^T [1024 d, 512 j]  f16
  wv    : W_g^T [1024 d, 512 j]  bf16
  wo    : Wo_g^T [512 c, 1024 j] bf16
  out   : OUT^T partial [1024 j, 2048 t] f32

The scalar engine's exp (256 ACTIVATEs of [128,1024], ~18.5us per
(pair,q-chunk)) is the co-critical resource next to the PE, so the schedule
is built around starting it early and never starving it: K proj, Q proj
(t-chunk 0 only needed), then QK blocks for q-chunk 0 begin immediately
while V proj and the remaining Q proj chunks fill the PE between them.
pt (exp output) is triple-buffered so QK(n) never waits on AV(n-2)'s
reads. Steady state runs a depth-3 software pipeline av(n-3); qk(n);
with out-proj for q-chunk q slotted after qk(4q+6) to cover the last
pair's normalization latency. Weight tiles for wk/wq/wv rotate through one
double-buffered slot (their lifetimes are disjoint); ct rotates per
q-chunk; x chunks load in half-d-blocks, triple buffered.
"""

import numpy as np
import ml_dtypes

D = 1024          # d_model
L = 2048          # sequence length
B = 4             # batch
HG = 512          # head-group width (8 heads x 64)
NCORES = 8
EXP_BIAS = -45.0  # softmax shift: exp(s-45); cancels in normalization

NT = 4            # token chunks of 512
TC = L // NT      # 512
NDB = D // 128    # 8 d-model blocks
NP = 4            # head pairs per group
NKB = L // 128    # 16 key blocks

_COMPILED = None
LAST_RESULT = None


def _build():
    import concourse.bacc as bacc
    import concourse.mybir as mybir
    import concourse.tile as tile

    f32 = mybir.dt.float32
    f16 = mybir.dt.float16
    bf16 = mybir.dt.bfloat16
    EXP = mybir.ActivationFunctionType.Exp
    MUL = mybir.AluOpType.mult

    nc = bacc.Bacc()

    xq = nc.declare_dram_parameter("xq", [D, L], f16, isOutput=False)
    xk = nc.declare_dram_parameter("xk", [D, L], f16, isOutput=False)
    xv = nc.declare_dram_parameter("xv", [D, L], bf16, isOutput=False)
    wq = nc.declare_dram_parameter("wq", [D, HG], f16, isOutput=False)
    wk = nc.declare_dram_parameter("wk", [D, HG], f16, isOutput=False)
    wv = nc.declare_dram_parameter("wv", [D, HG], bf16, isOutput=False)
    wo = nc.declare_dram_parameter("wo", [HG, D], bf16, isOutput=False)
    bq = nc.declare_dram_parameter("bq", [HG], f32, isOutput=False)
    out = nc.declare_dram_parameter("out", [D, L], f32, isOutput=True)

    out_v = out.rearrange("(ob p) (n t) -> ob p n t", p=128, t=TC)

    with tile.TileContext(nc) as tc:
        with tc.tile_pool(name="res", bufs=1) as res, tc.tile_pool(
            name="pa", bufs=1
        ) as pa, tc.tile_pool(name="psum", bufs=1, space="PSUM") as psum:
            # ---- resident tiles ----
            kt_sb = res.tile([128, NP, L], f16)
            qt_sb = res.tile([128, NP, L], f16)
            wo_sb = res.tile([128, NP, D], bf16)
            bq_sb = res.tile([128, NP], f32)
            bias_exp = res.tile([128, 1], f32)
            nc.vector.memset(bias_exp[:], EXP_BIAS)

            # V stationary: per (kb, pair): [V_even(64) | 1 | V_odd(64) | 1]
            # even AV uses cols 0:65, odd cols 65:130; both land at PSUM
            # partitions 0:65 with the rowsum at partition 64.
            v_sb = res.tile([128, NKB, NP, 130], bf16)
            nc.vector.memset(v_sb[:, :, :, 64:65], 1.0)
            nc.vector.memset(v_sb[:, :, :, 129:130], 1.0)

            def wtile(dt):
                return pa.tile([128, NDB, HG], dt, name="w", tag="w", bufs=2)

            _xeng = [0]

            def load_x_half(src, t, h, dt, eng=None):
                # loop-phase DMAs alternate sync/gpsimd only: an issue on the
                # scalar engine would queue in front of later ACTIVATEs and a
                # WAR-gated one blocks the whole exp stream behind it
                if eng is None:
                    eng = (nc.sync, nc.gpsimd)[_xeng[0] % 2]
                    _xeng[0] += 1
                xt = pa.tile([128, NDB // 2, TC], dt, name="xt", tag="xt", bufs=3)
                eng.dma_start(
                    out=xt[:],
                    in_=src.rearrange("(db p) (n t) -> p db n t", p=128, t=TC)[
                        :, 4 * h : 4 * h + 4, t
                    ],
                )
                return xt

            def emit_kqproj(src, w_sb, t, is_q, xh=None):
                if xh is None:
                    xh = [load_x_half(src, t, h, f16) for h in range(2)]
                for jb in range(NP):
                    ps = psum.tile([128, TC], f32, name="acc", tag="accu", bufs=2)
                    for db in range(NDB):
                        nc.tensor.matmul(
                            ps[:],
                            w_sb[:, db, jb * 128 : (jb + 1) * 128],
                            xh[db // 4][:, db % 4, :],
                            start=(db == 0),
                            stop=(db == NDB - 1),
                        )
                    if is_q:
                        nc.vector.tensor_scalar_add(
                            qt_sb[:, jb, t * TC : (t + 1) * TC],
                            ps[:],
                            bq_sb[:, jb : jb + 1],
                        )
                    else:
                        nc.vector.tensor_copy(
                            kt_sb[:, jb, t * TC : (t + 1) * TC], ps[:]
                        )

            def emit_vproj(wv_sb, t):
                xh = [load_x_half(xv, t, h, bf16) for h in range(2)]
                for tb in range(4):
                    kb = t * 4 + tb
                    ps = psum.tile(
                        [128, NP, 128], f32, name="acc", tag="accu", bufs=2
                    )
                    for db in range(NDB):
                        nc.tensor.matmul(
                            ps[:],
                            xh[db // 4][:, db % 4, tb * 128 : (tb + 1) * 128],
                            wv_sb[:, db, :],
                            start=(db == 0),
                            stop=(db == NDB - 1),
                        )
                    nc.vector.tensor_copy(v_sb[:, kb, :, 0:64], ps[:, :, 0:64])
                    nc.vector.tensor_copy(v_sb[:, kb, :, 65:129], ps[:, :, 64:128])

            def new_pt():
                return pa.tile([128, NKB, 2, TC], bf16, name="pt", tag="pt", bufs=3)

            def emit_qk_kbs(p, q, pt, kbs):
                qsl = slice(q * TC, (q + 1) * TC)
                for kb in kbs:
                    ps_s = psum.tile(
                        [128, 2, TC], f32, name="ps_s", tag="ps_s", bufs=2
                    )
                    nc.tensor.matmul(
                        ps_s[:, 0, :],
                        kt_sb[0:64, p, kb * 128 : (kb + 1) * 128],
                        qt_sb[0:64, p, qsl],
                        start=True,
                        stop=True,
                    )
                    nc.tensor.matmul(
                        ps_s[:, 1, :],
                        kt_sb[64:128, p, kb * 128 : (kb + 1) * 128],
                        qt_sb[64:128, p, qsl],
                        start=True,
                        stop=True,
                    )
                    nc.scalar.activation(
                        pt[:, kb, :, :], ps_s[:], EXP, bias=bias_exp[:], scale=1.0
                    )

            def emit_qk(p, q):
                pt = new_pt()
                emit_qk_kbs(p, q, pt, range(NKB))
                return pt

            def emit_av(p, ct, pt):
                ps_u = psum.tile([128, 2, TC], f32, name="ps_u", tag="accu", bufs=2)
                for kb in range(NKB):
                    nc.tensor.matmul(
                        ps_u[0:65, 0, :],
                        v_sb[:, kb, p, 0:65],
                        pt[:, kb, 0, :],
                        start=(kb == 0),
                        stop=(kb == NKB - 1),
                    )
                    nc.tensor.matmul(
                        ps_u[0:65, 1, :],
                        v_sb[:, kb, p, 65:130],
                        pt[:, kb, 1, :],
                        start=(kb == 0),
                        stop=(kb == NKB - 1),
                    )
                # normalize: ct = U^T * (1/r); odd head computed at 0:64 then
                # DMA-shifted to partitions 64:128
                # custom-DVE ops misread PSUM: bounce the rowsum row to SBUF
                # first, then approx-recip (regular reciprocal is ~6.5us/row)
                rr_raw = pa.tile([1, 2, TC], f32, name="rr_raw", tag="rrw", bufs=1)
                nc.vector.tensor_copy(rr_raw[:], ps_u[64:65, :, :])
                rr = pa.tile([1, 2, TC], f32, name="rr", tag="rr", bufs=1)
                nc.vector.reciprocal_approx_fast(rr[:], rr_raw[:])
                rb = pa.tile([64, 2, TC], f32, name="rb", tag="rb", bufs=1)
                nc.gpsimd.partition_broadcast(rb[:, 0, :], rr[:, 0, :], channels=64)
                nc.gpsimd.partition_broadcast(rb[:, 1, :], rr[:, 1, :], channels=64)
                nc.vector.tensor_tensor(
                    out=ct[0:64, p, :],
                    in0=ps_u[0:64, 0, :],
                    in1=rb[:, 0, :],
                    op=MUL,
                )
                ct_o = pa.tile([64, TC], bf16, name="ct_o", tag="ct_o", bufs=1)
                nc.vector.tensor_tensor(
                    out=ct_o[:], in0=ps_u[0:64, 1, :], in1=rb[:, 1, :], op=MUL
                )
                nc.sync.dma_start(out=ct[64:128, p, :], in_=ct_o[:])

            def emit_outproj(q, ct):
                for ob in range(NDB):
                    ps = psum.tile([128, TC], f32, name="po", tag="accu", bufs=2)
                    for p in range(NP):
                        nc.tensor.matmul(
                            ps[:],
                            wo_sb[:, p, ob * 128 : (ob + 1) * 128],
                            ct[:, p, :],
                            start=(p == 0),
                            stop=(p == NP - 1),
                        )
                    o_sb = pa.tile([128, TC], f32, name="o_sb", tag="o_sb", bufs=3)
                    nc.vector.tensor_copy(o_sb[:], ps[:])
                    (nc.sync, nc.gpsimd)[ob % 2].dma_start(
                        out=out_v[ob, :, q], in_=o_sb[:]
                    )

            # ---- schedule ----
            # qk(0,0) is burst-interleaved with the K projection at kb
            # granularity (kb 4t..4t+3 only needs K proj chunk t), so the
            # scalar engine's exp stream starts ~19us into the kernel.
            # Startup DMAs are choreographed across the three DMA-capable
            # engines' issue queues. Weight loads split along db (1KB
            # contiguous rows — j-splits give 256B elements and multi-us
            # issue times). x halves go first in each queue they share.
            # The scalar engine gets only ungated startup loads: anything
            # WAR-gated would block the exp stream behind it.
            def load_w_piece(w_sb, src, dq, eng):
                wr = src.rearrange("(db p) j -> p db j", p=128)
                eng.dma_start(
                    out=w_sb[:, 2 * dq : 2 * dq + 2, :],
                    in_=wr[:, 2 * dq : 2 * dq + 2, :],
                )

            xk0 = [
                load_x_half(xk, 0, 0, f16, nc.gpsimd),
                load_x_half(xk, 0, 1, f16, nc.scalar),
            ]
            wk_sb = wtile(f16)
            for dq, eng in enumerate([nc.sync, nc.sync, nc.gpsimd, nc.scalar]):
                load_w_piece(wk_sb, wk, dq, eng)
            xq0 = [
                load_x_half(xq, 0, 0, f16, nc.gpsimd),
                load_x_half(xq, 0, 1, f16, nc.scalar),
            ]
            wq_sb = wtile(f16)
            for dq, eng in enumerate([nc.sync, nc.gpsimd, nc.scalar, nc.sync]):
                load_w_piece(wq_sb, wq, dq, eng)
            nc.scalar.dma_start(
                out=bq_sb[:], in_=bq.rearrange("(jb p) -> p jb", p=128)
            )
            emit_kqproj(xk, wk_sb, 0, False, xh=xk0)
            emit_kqproj(xq, wq_sb, 0, True, xh=xq0)

            pts = {}
            cts = {}

            def new_ct():
                return pa.tile([128, NP, TC], bf16, name="ct", tag="ct", bufs=2)

            pts[0] = new_pt()
            emit_qk_kbs(0, 0, pts[0], range(0, 4))
            for t in range(1, NT):
                emit_kqproj(xk, wk_sb, t, False)
                emit_qk_kbs(0, 0, pts[0], range(4 * t, 4 * t + 4))
            pts[1] = emit_qk(1, 0)
            wv_sb = wtile(bf16)
            nc.sync.dma_start(
                out=wv_sb[:], in_=wv.rearrange("(db p) j -> p db j", p=128)
            )
            emit_vproj(wv_sb, 0)
            emit_vproj(wv_sb, 1)
            pts[2] = emit_qk(2, 0)
            emit_vproj(wv_sb, 2)
            emit_vproj(wv_sb, 3)
            nc.sync.dma_start(
                out=wo_sb[:], in_=wo.rearrange("(cb p) j -> p cb j", p=128)
            )

            # depth-3 pipeline: av(n-3); qk(n). Q proj chunks 1-3 ride along as
            # PE fillers, one per q-cycle (qproj(i) just before qk(4i) needs it)
            # so each 4-pair cycle is PE 68us vs ACT 74us — ACT stays pacer.
            for n in range(3, 16):
                m = n - 3
                if m % 4 == 0:
                    cts[m // 4] = new_ct()
                emit_av(m % 4, cts[m // 4], pts.pop(m))
                pts[n] = emit_qk(n % 4, n // 4)
                # fillers go AFTER qk(n): exp(n) is gated on qk(n)'s matmuls,
                # so PE filler work placed before qk(n) starves the exp stream
                if n in (3, 7, 11):
                    emit_kqproj(xq, wq_sb, (n - 3) // 4 + 1, True)
                if n % 4 == 2 and n > 3:
                    emit_outproj(n // 4 - 1, cts[n // 4 - 1])
            for m in (13, 14, 15):
                if m % 4 == 0:
                    cts[m // 4] = new_ct()
                emit_av(m % 4, cts[m // 4], pts.pop(m))
            emit_outproj(3, cts[3])

    nc.compile()
    return nc


def _get_compiled():
    global _COMPILED
    if _COMPILED is None:
        _COMPILED = _build()
    return _COMPILED


def kernel(q, k, v, Wq, bq, Wk, bk, Wv, bv, Wo, bo):
    global LAST_RESULT
    from concourse.bass_utils import run_bass_kernel_spmd

    nc = _get_compiled()

    f16 = np.float16
    bf = ml_dtypes.bfloat16
    q = np.asarray(q, dtype=np.float32)
    k = np.asarray(k, dtype=np.float32)
    v = np.asarray(v, dtype=np.float32)
    Wq = np.asarray(Wq, dtype=np.float32)
    Wk = np.asarray(Wk, dtype=np.float32)
    Wv = np.asarray(Wv, dtype=np.float32)
    Wo = np.asarray(Wo, dtype=np.float32)
    bq = np.asarray(bq, dtype=np.float32)
    bv = np.asarray(bv, dtype=np.float32)
    bo = np.asarray(bo, dtype=np.float32)

    xT = {}
    for b in range(B):
        xT[("q", b)] = np.ascontiguousarray(q[b].T).astype(f16)
        xT[("k", b)] = np.ascontiguousarray(k[b].T).astype(f16)
        xT[("v", b)] = np.ascontiguousarray(v[b].T).astype(bf)

    wqT = [
        np.ascontiguousarray(Wq[g * HG : (g + 1) * HG, :].T).astype(f16)
        for g in range(2)
    ]
    wkT = [
        np.ascontiguousarray(Wk[g * HG : (g + 1) * HG, :].T).astype(f16)
        for g in range(2)
    ]
    wvT = [
        np.ascontiguousarray(Wv[g * HG : (g + 1) * HG, :].T).astype(bf)
        for g in range(2)
    ]
    woT = [
        np.ascontiguousarray(Wo[:, g * HG : (g + 1) * HG].T).astype(bf)
        for g in range(2)
    ]
    bqg = [np.ascontiguousarray(bq[g * HG : (g + 1) * HG]) for g in range(2)]

    in_maps = []
    for core in range(NCORES):
        b, g = core // 2, core % 2
        in_maps.append(
            {
                "xq": xT[("q", b)],
                "xk": xT[("k", b)],
                "xv": xT[("v", b)],
                "wq": wqT[g],
                "wk": wkT[g],
                "wv": wvT[g],
                "wo": woT[g],
                "bq": bqg[g],
            }
        )

    res = run_bass_kernel_spmd(nc, in_maps, core_ids=list(range(NCORES)))
    LAST_RESULT = res

    # bv never enters the device kernel: sum_k P_k bv = r * bv, so after
    # normalization it contributes exactly bv per token -> Wo @ bv on host.
    const = bo + Wo @ bv
    outp = np.empty((B, L, D), dtype=np.float32)
    for b in range(B):
        acc = res.results[2 * b]["out"].T + res.results[2 * b + 1]["out"].T
        outp[b] = acc + const
    return outp


# revision 36
# speedup vs baseline: 1.1429x; 1.0154x over previous
"""Multi-head attention TRN2 kernel: 8 cores = 4 batch x 2 head-groups.

Per core (b = core//2, g = core%2): attention for batch b, heads [8g, 8g+8),
producing the transposed partial output projection. Host sums the two
head-group partials per batch and adds (bo + Wo @ bv) once.

Precision: fp16 for the Q/K chain (x, W, Q^T, K^T, QK matmul — 10-bit
mantissa keeps score error small on std-8 scores), bf16 for the V/P/C/Wo
chain (range needed for exp(s-45) ~ 1e-20..1e16). PSUM f32.

Layouts (per core, host-prepped):
  xq/xk : x^T   [1024 d, 2048 t] f16
  xv    : x^T   [1024 d, 2048 t] bf16
  wq/wk : W_g^T [1024 d, 512 j]  f16
  wv    : W_g^T [1024 d, 512 j]  bf16
  wo    : Wo_g^T [512 c, 1024 j] bf16
  out   : OUT^T partial [1024 j, 2048 t] f32

The scalar engine's exp (256 ACTIVATEs of [128,1024], ~18.5us per
(pair,q-chunk)) is the co-critical resource next to the PE, so the schedule
is built around starting it early and never starving it: K proj, Q proj
(t-chunk 0 only needed), then QK blocks for q-chunk 0 begin immediately
while V proj and the remaining Q proj chunks fill the PE between them.
pt (exp output) is triple-buffered so QK(n) never waits on AV(n-2)'s
reads. Steady state runs a depth-3 software pipeline av(n-3); qk(n);
with out-proj for q-chunk q slotted after qk(4q+6) to cover the last
pair's normalization latency. Weight tiles for wk/wq/wv rotate through one
double-buffered slot (their lifetimes are disjoint); ct rotates per
q-chunk; x chunks load in half-d-blocks, triple buffered.
"""

import numpy as np
import ml_dtypes

D = 1024          # d_model
L = 2048          # sequence length
B = 4             # batch
HG = 512          # head-group width (8 heads x 64)
NCORES = 8
EXP_BIAS = -45.0  # softmax shift: exp(s-45); cancels in normalization

NT = 4            # token chunks of 512
TC = L // NT      # 512
NDB = D // 128    # 8 d-model blocks
NP = 4            # head pairs per group
NKB = L // 128    # 16 key blocks

_COMPILED = None
LAST_RESULT = None


def _build():
    import concourse.bacc as bacc
    import concourse.mybir as mybir
    import concourse.tile as tile

    f32 = mybir.dt.float32
    f16 = mybir.dt.float16
    bf16 = mybir.dt.bfloat16
    EXP = mybir.ActivationFunctionType.Exp
    MUL = mybir.AluOpType.mult

    nc = bacc.Bacc()

    xq = nc.declare_dram_parameter("xq", [D, L], f16, isOutput=False)
    xk = nc.declare_dram_parameter("xk", [D, L], f16, isOutput=False)
    xv = nc.declare_dram_parameter("xv", [D, L], bf16, isOutput=False)
    wq = nc.declare_dram_parameter("wq", [D, HG], f16, isOutput=False)
    wk = nc.declare_dram_parameter("wk", [D, HG], f16, isOutput=False)
    wv = nc.declare_dram_parameter("wv", [D, HG], bf16, isOutput=False)
    wo = nc.declare_dram_parameter("wo", [HG, D], bf16, isOutput=False)
    bq = nc.declare_dram_parameter("bq", [HG], f32, isOutput=False)
    out = nc.declare_dram_parameter("out", [D, L], f32, isOutput=True)

    out_v = out.rearrange("(ob p) (n t) -> ob p n t", p=128, t=TC)

    with tile.TileContext(nc) as tc:
        with tc.tile_pool(name="res", bufs=1) as res, tc.tile_pool(
            name="pa", bufs=1
        ) as pa, tc.tile_pool(name="psum", bufs=1, space="PSUM") as psum:
            # ---- resident tiles ----
            kt_sb = res.tile([128, NP, L], f16)
            qt_sb = res.tile([128, NP, L], f16)
            wo_sb = res.tile([128, NP, D], bf16)
            bq_sb = res.tile([128, NP], f32)
            bias_exp = res.tile([128, 1], f32)
            nc.vector.memset(bias_exp[:], EXP_BIAS)

            # V stationary: per (kb, pair): [V_even(64) | 1 | V_odd(64) | 1]
            # even AV uses cols 0:65, odd cols 65:130; both land at PSUM
            # partitions 0:65 with the rowsum at partition 64.
            v_sb = res.tile([128, NKB, NP, 130], bf16)
            nc.vector.memset(v_sb[:, :, :, 64:65], 1.0)
            nc.vector.memset(v_sb[:, :, :, 129:130], 1.0)

            def wtile(dt):
                return pa.tile([128, NDB, HG], dt, name="w", tag="w", bufs=2)

            _xeng = [0]

            def load_x_half(src, t, h, dt, eng=None):
                # loop-phase DMAs alternate sync/gpsimd only: an issue on the
                # scalar engine would queue in front of later ACTIVATEs and a
                # WAR-gated one blocks the whole exp stream behind it
                if eng is None:
                    eng = (nc.sync, nc.gpsimd)[_xeng[0] % 2]
                    _xeng[0] += 1
                xt = pa.tile([128, NDB // 2, TC], dt, name="xt", tag="xt", bufs=3)
                eng.dma_start(
                    out=xt[:],
                    in_=src.rearrange("(db p) (n t) -> p db n t", p=128, t=TC)[
                        :, 4 * h : 4 * h + 4, t
                    ],
                )
                return xt

            def emit_kqproj(src, w_sb, t, is_q, xh=None):
                if xh is None:
                    xh = [load_x_half(src, t, h, f16) for h in range(2)]
                for jb in range(NP):
                    ps = psum.tile([128, TC], f32, name="acc", tag="accu", bufs=2)
                    for db in range(NDB):
                        nc.tensor.matmul(
                            ps[:],
                            w_sb[:, db, jb * 128 : (jb + 1) * 128],
                            xh[db // 4][:, db % 4, :],
                            start=(db == 0),
                            stop=(db == NDB - 1),
                        )
                    if is_q:
                        nc.vector.tensor_scalar_add(
                            qt_sb[:, jb, t * TC : (t + 1) * TC],
                            ps[:],
                            bq_sb[:, jb : jb + 1],
                        )
                    else:
                        nc.vector.tensor_copy(
                            kt_sb[:, jb, t * TC : (t + 1) * TC], ps[:]
                        )

            def emit_vproj(wv_sb, t):
                xh = [load_x_half(xv, t, h, bf16) for h in range(2)]
                for tb in range(4):
                    kb = t * 4 + tb
                    ps = psum.tile(
                        [128, NP, 128], f32, name="acc", tag="accu", bufs=2
                    )
                    for db in range(NDB):
                        nc.tensor.matmul(
                            ps[:],
                            xh[db // 4][:, db % 4, tb * 128 : (tb + 1) * 128],
                            wv_sb[:, db, :],
                            start=(db == 0),
                            stop=(db == NDB - 1),
                        )
                    nc.vector.tensor_copy(v_sb[:, kb, :, 0:64], ps[:, :, 0:64])
                    nc.vector.tensor_copy(v_sb[:, kb, :, 65:129], ps[:, :, 64:128])

            def new_pt():
                return pa.tile([128, NKB, 2, TC], bf16, name="pt", tag="pt", bufs=3)

            def emit_qk_kbs(p, q, pt, kbs):
                qsl = slice(q * TC, (q + 1) * TC)
                for kb in kbs:
                    ps_s = psum.tile(
                        [128, 2, TC], f32, name="ps_s", tag="ps_s", bufs=2
                    )
                    nc.tensor.matmul(
                        ps_s[:, 0, :],
                        kt_sb[0:64, p, kb * 128 : (kb + 1) * 128],
                        qt_sb[0:64, p, qsl],
                        start=True,
                        stop=True,
                    )
                    nc.tensor.matmul(
                        ps_s[:, 1, :],
                        kt_sb[64:128, p, kb * 128 : (kb + 1) * 128],
                        qt_sb[64:128, p, qsl],
                        start=True,
                        stop=True,
                    )
                    nc.scalar.activation(
                        pt[:, kb, :, :], ps_s[:], EXP, bias=bias_exp[:], scale=1.0
                    )

            def emit_qk(p, q):
                pt = new_pt()
                emit_qk_kbs(p, q, pt, range(NKB))
                return pt

            def emit_av(p, ct, pt):
                ps_u = psum.tile([128, 2, TC], f32, name="ps_u", tag="accu", bufs=2)
                for kb in range(NKB):
                    nc.tensor.matmul(
                        ps_u[0:65, 0, :],
                        v_sb[:, kb, p, 0:65],
                        pt[:, kb, 0, :],
                        start=(kb == 0),
                        stop=(kb == NKB - 1),
                    )
                    nc.tensor.matmul(
                        ps_u[0:65, 1, :],
                        v_sb[:, kb, p, 65:130],
                        pt[:, kb, 1, :],
                        start=(kb == 0),
                        stop=(kb == NKB - 1),
                    )
                # normalize: ct = U^T * (1/r); odd head computed at 0:64 then
                # DMA-shifted to partitions 64:128
                # custom-DVE ops misread PSUM: bounce the rowsum row to SBUF
                # first, then approx-recip (regular reciprocal is ~6.5us/row)
                rr_raw = pa.tile([1, 2, TC], f32, name="rr_raw", tag="rrw", bufs=1)
                nc.vector.tensor_copy(rr_raw[:], ps_u[64:65, :, :])
                rr = pa.tile([1, 2, TC], f32, name="rr", tag="rr", bufs=1)
                nc.vector.reciprocal_approx_fast(rr[:], rr_raw[:])
                rb = pa.tile([64, 2, TC], f32, name="rb", tag="rb", bufs=1)
                nc.gpsimd.partition_broadcast(rb[:, 0, :], rr[:, 0, :], channels=64)
                nc.gpsimd.partition_broadcast(rb[:, 1, :], rr[:, 1, :], channels=64)
                nc.vector.tensor_tensor(
                    out=ct[0:64, p, :],
                    in0=ps_u[0:64, 0, :],
                    in1=rb[:, 0, :],
                    op=MUL,
                )
                ct_o = pa.tile([64, TC], bf16, name="ct_o", tag="ct_o", bufs=1)
                nc.vector.tensor_tensor(
                    out=ct_o[:], in0=ps_u[0:64, 1, :], in1=rb[:, 1, :], op=MUL
                )
                nc.sync.dma_start(out=ct[64:128, p, :], in_=ct_o[:])

            def emit_outproj(q, ct, obs=range(NDB)):
                for ob in obs:
                    ps = psum.tile([128, TC], f32, name="po", tag="accu", bufs=2)
                    for p in range(NP):
                        nc.tensor.matmul(
                            ps[:],
                            wo_sb[:, p, ob * 128 : (ob + 1) * 128],
                            ct[:, p, :],
                            start=(p == 0),
                            stop=(p == NP - 1),
                        )
                    o_sb = pa.tile([128, TC], f32, name="o_sb", tag="o_sb", bufs=3)
                    nc.vector.tensor_copy(o_sb[:], ps[:])
                    (nc.sync, nc.gpsimd)[ob % 2].dma_start(
                        out=out_v[ob, :, q], in_=o_sb[:]
                    )

            # ---- schedule ----
            # qk(0,0) is burst-interleaved with the K projection at kb
            # granularity (kb 4t..4t+3 only needs K proj chunk t), so the
            # scalar engine's exp stream starts ~19us into the kernel.
            # Startup DMAs are choreographed across the three DMA-capable
            # engines' issue queues. Weight loads split along db (1KB
            # contiguous rows — j-splits give 256B elements and multi-us
            # issue times). x halves go first in each queue they share.
            # The scalar engine gets only ungated startup loads: anything
            # WAR-gated would block the exp stream behind it.
            def load_w_piece(w_sb, src, dq, eng):
                wr = src.rearrange("(db p) j -> p db j", p=128)
                eng.dma_start(
                    out=w_sb[:, 2 * dq : 2 * dq + 2, :],
                    in_=wr[:, 2 * dq : 2 * dq + 2, :],
                )

            xk0 = [
                load_x_half(xk, 0, 0, f16, nc.gpsimd),
                load_x_half(xk, 0, 1, f16, nc.scalar),
            ]
            wk_sb = wtile(f16)
            for dq, eng in enumerate([nc.sync, nc.sync, nc.gpsimd, nc.scalar]):
                load_w_piece(wk_sb, wk, dq, eng)
            xq0 = [
                load_x_half(xq, 0, 0, f16, nc.gpsimd),
                load_x_half(xq, 0, 1, f16, nc.scalar),
            ]
            wq_sb = wtile(f16)
            for dq, eng in enumerate([nc.sync, nc.gpsimd, nc.scalar, nc.sync]):
                load_w_piece(wq_sb, wq, dq, eng)
            nc.scalar.dma_start(
                out=bq_sb[:], in_=bq.rearrange("(jb p) -> p jb", p=128)
            )
            emit_kqproj(xk, wk_sb, 0, False, xh=xk0)
            emit_kqproj(xq, wq_sb, 0, True, xh=xq0)

            pts = {}
            cts = {}

            def new_ct():
                return pa.tile([128, NP, TC], bf16, name="ct", tag="ct", bufs=2)

            pts[0] = new_pt()
            emit_qk_kbs(0, 0, pts[0], range(0, 4))
            for t in range(1, NT):
                emit_kqproj(xk, wk_sb, t, False)
                emit_qk_kbs(0, 0, pts[0], range(4 * t, 4 * t + 4))
            pts[1] = emit_qk(1, 0)
            wv_sb = wtile(bf16)
            nc.sync.dma_start(
                out=wv_sb[:], in_=wv.rearrange("(db p) j -> p db j", p=128)
            )
            emit_vproj(wv_sb, 0)
            emit_vproj(wv_sb, 1)
            pts[2] = emit_qk(2, 0)
            emit_vproj(wv_sb, 2)
            emit_vproj(wv_sb, 3)
            nc.sync.dma_start(
                out=wo_sb[:], in_=wo.rearrange("(cb p) j -> p cb j", p=128)
            )

            # depth-3 pipeline: av(n-3); qk(n). Q proj chunks 1-3 ride along as
            # PE fillers, one per q-cycle (qproj(i) just before qk(4i) needs it)
            # so each 4-pair cycle is PE 68us vs ACT 74us — ACT stays pacer.
            # Fillers and out-proj halves go AFTER qk(n) — exp(n) is gated on
            # qk(n)'s matmuls, so extra PE work before qk(n) starves the exp
            # stream — and are spread so no loop step exceeds ~20us of PE work
            # against ACT's 18.5us/pair pace. outproj(2) moves into the tail
            # to keep the PE hot (full pstate) while the last exps drain.
            fillers = {3: 1, 5: 2, 9: 3}
            for n in range(3, 16):
                m = n - 3
                if m % 4 == 0:
                    cts[m // 4] = new_ct()
                emit_av(m % 4, cts[m // 4], pts.pop(m))
                pts[n] = emit_qk(n % 4, n // 4)
                if n in fillers:
                    emit_kqproj(xq, wq_sb, fillers[n], True)
                if n in (6, 7):
                    emit_outproj(0, cts[0], range(4 * (n - 6), 4 * (n - 6) + 4))
                if n in (10, 11):
                    emit_outproj(1, cts[1], range(4 * (n - 10), 4 * (n - 10) + 4))
            emit_av(1, cts[3], pts.pop(13))
            emit_outproj(2, cts[2])
            emit_av(2, cts[3], pts.pop(14))
            emit_av(3, cts[3], pts.pop(15))
            emit_outproj(3, cts[3])

    nc.compile()
    return nc


def _get_compiled():
    global _COMPILED
    if _COMPILED is None:
        _COMPILED = _build()
    return _COMPILED


def kernel(q, k, v, Wq, bq, Wk, bk, Wv, bv, Wo, bo):
    global LAST_RESULT
    from concourse.bass_utils import run_bass_kernel_spmd

    nc = _get_compiled()

    f16 = np.float16
    bf = ml_dtypes.bfloat16
    q = np.asarray(q, dtype=np.float32)
    k = np.asarray(k, dtype=np.float32)
    v = np.asarray(v, dtype=np.float32)
    Wq = np.asarray(Wq, dtype=np.float32)
    Wk = np.asarray(Wk, dtype=np.float32)
    Wv = np.asarray(Wv, dtype=np.float32)
    Wo = np.asarray(Wo, dtype=np.float32)
    bq = np.asarray(bq, dtype=np.float32)
    bv = np.asarray(bv, dtype=np.float32)
    bo = np.asarray(bo, dtype=np.float32)

    xT = {}
    for b in range(B):
        xT[("q", b)] = np.ascontiguousarray(q[b].T).astype(f16)
        xT[("k", b)] = np.ascontiguousarray(k[b].T).astype(f16)
        xT[("v", b)] = np.ascontiguousarray(v[b].T).astype(bf)

    wqT = [
        np.ascontiguousarray(Wq[g * HG : (g + 1) * HG, :].T).astype(f16)
        for g in range(2)
    ]
    wkT = [
        np.ascontiguousarray(Wk[g * HG : (g + 1) * HG, :].T).astype(f16)
        for g in range(2)
    ]
    wvT = [
        np.ascontiguousarray(Wv[g * HG : (g + 1) * HG, :].T).astype(bf)
        for g in range(2)
    ]
    woT = [
        np.ascontiguousarray(Wo[:, g * HG : (g + 1) * HG].T).astype(bf)
        for g in range(2)
    ]
    bqg = [np.ascontiguousarray(bq[g * HG : (g + 1) * HG]) for g in range(2)]

    in_maps = []
    for core in range(NCORES):
        b, g = core // 2, core % 2
        in_maps.append(
            {
                "xq": xT[("q", b)],
                "xk": xT[("k", b)],
                "xv": xT[("v", b)],
                "wq": wqT[g],
                "wk": wkT[g],
                "wv": wvT[g],
                "wo": woT[g],
                "bq": bqg[g],
            }
        )

    res = run_bass_kernel_spmd(nc, in_maps, core_ids=list(range(NCORES)))
    LAST_RESULT = res

    # bv never enters the device kernel: sum_k P_k bv = r * bv, so after
    # normalization it contributes exactly bv per token -> Wo @ bv on host.
    const = bo + Wo @ bv
    outp = np.empty((B, L, D), dtype=np.float32)
    for b in range(B):
        acc = res.results[2 * b]["out"].T + res.results[2 * b + 1]["out"].T
        outp[b] = acc + const
    return outp
